# revision 11
# baseline (speedup 1.0000x reference)
"""A^2-style sparse attention (double-attention) kernel for 8 Trainium2 cores.

Computation (per the reference):
  x = concat(content, style)              (2, 256, 256, 256) -> (256, S) with S = 131072
  theta = Wt x + bt ; pi = Wp x + bp ; g = Wg x + bg      (128, S) each
  softmax_pi = softmax(pi, axis=S per channel)            (gathering weights)
  softmax_g  = softmax(g, axis=channels per pixel)        (distribution weights)
  G = theta @ softmax_pi^T                                (128, 128)
  Z = G @ softmax_g                                       (128, S)
  out = Wo Z + bo                                         (256, S)

Sharding: spatial axis S split across 8 cores (16384 columns each).
Algebraic restructuring used on device (validated vs reference to ~4e-7):
  * softmax_pi is invariant to the per-row bias bp -> bp dropped entirely.
  * no max subtraction in either softmax (|pi|,|g| <= ~3.3; exp is safe).
  * pi/theta are computed directly TRANSPOSED, (s, n)-layout, by using the
    x chunk as the matmul stationary operand: piT = x_chunk^T @ WpT.
  * G_raw[m,n] = sum_s thetaT_nb[s,m] e[s,n] accumulated in PSUM; the
    theta bias folds to G += bt x 1^T which (softmax sums to 1) becomes
    Z += bt, applied as the Z PSUM-evacuation bias.
  * cross-core reduction: single 129x128 AllReduce of [G_raw ; r] where
    r[n] = sum_s exp(pi[s,n]).
  * alpha_gathering = e * (1/r) broadcast per column (gpsimd row-broadcast).
  * g softmax runs in natural (n, s) layout; per-pixel sums over channels
    come from a ones-vector matmul (partition reduction on the PE).
"""

import numpy as np

import concourse.bass as bass
import concourse.tile as tile
from concourse import mybir
from concourse.bass_utils import run_bass_kernel_spmd
from concourse.masks import make_identity
from concourse.vector_clock import ScopedClock

f32 = mybir.dt.float32
AF = mybir.ActivationFunctionType

C_IN = 256
C_ATT = 128
K = 2
H = W = 256
HW = H * W
S = K * HW
N_CORES = 8
SSH = S // N_CORES          # 16384 spatial columns per core
NSUB = SSH // 128           # 128 transposed subchunks per core
NCH = SSH // 512            # 32 natural 512-chunks per core
DCH = 1024                  # DMA chunk (columns)
NDMA = SSH // DCH           # 16 input DMA chunks


# ---------------------------------------------------------------------------
# Workaround: walrus in this container rejects >1 sem wait on a sync-engine
# CTRL instruction.  Split the TileContext exit-drain waits across nops.
def _patched_drain_and_barrier(self, tick_clock, wait_clock):
    nc = self.nc
    absorbers = [nc.sync.nop(nofuse=True) for _ in range(32)]
    drain_inst = nc.sync.drain()
    wait_clock.add_sem_waits(
        drain_inst.ins, ScopedClock({None: tick_clock.global_clock})
    )
    si = drain_inst.ins.sync_info
    waits = list(si.on_wait) if (si is not None and si.on_wait) else []
    if len(waits) > 1:
        si.on_wait = waits[-1:]
        for ab, w in zip(absorbers, waits[:-1]):
            asi = ab.ins.sync_info
            if asi is None:
                ab.ins.sync_info = type(si)(on_wait=[w], on_update=[])
            else:
                asi.on_wait = list(asi.on_wait or []) + [w]
    nc.all_engine_barrier()
    assert self.sems is not None
    popped = nc._tile_sem_poison_stack.pop()
    assert popped is self._sem_poison
    nc.clear_and_free_semaphores(list(self.sems.allocated().values()))
    nc.all_engine_barrier()


tile.TileContext._drain_and_barrier = _patched_drain_and_barrier


def _split_excess_waits(nc, limit=1):
    """Walrus here accepts only `limit` sem-waits per instruction; move any
    excess waits onto same-engine nops inserted immediately before."""
    n_fix = 0
    for bb in nc.main_func.blocks:
        out = []
        for ins in bb.instructions:
            si = ins.sync_info
            waits = list(si.on_wait) if (si is not None and si.on_wait) else []
            if len(waits) > limit:
                si.on_wait = waits[-limit:]
                for w in waits[:-limit]:
                    n_fix += 1
                    nop = mybir.InstNoOp(
                        name=f"I-waitfix-{n_fix}",
                        engine=ins.engine,
                        sync_info=mybir.SyncInfo(on_wait=[w], on_update=[]),
                    )
                    out.append(nop)
            out.append(ins)
        bb.instructions = out
    return n_fix
# ---------------------------------------------------------------------------


def _build_nc():
    nc = bass.Bass()

    # ---- per-core DRAM I/O ----
    x_d = nc.dram_tensor("x", [C_IN, SSH], f32, kind="ExternalInput")
    wpt_d = nc.dram_tensor("wpt", [128, 2, 256], f32, kind="ExternalInput")
    wg_d = nc.dram_tensor("wg", [128, 2, 128], f32, kind="ExternalInput")
    wout_d = nc.dram_tensor("wout", [128, 2, 128], f32, kind="ExternalInput")
    bg_d = nc.dram_tensor("bg", [128, 1], f32, kind="ExternalInput")
    bt_d = nc.dram_tensor("bt", [128, 1], f32, kind="ExternalInput")
    bo_d = nc.dram_tensor("bo", [128, 2], f32, kind="ExternalInput")

    out_d = nc.dram_tensor("out", [128, 2, SSH], f32, kind="ExternalOutput")
    ag_d = nc.dram_tensor("ag", [NSUB, 128, 128], f32, kind="ExternalOutput")
    ad_d = nc.dram_tensor("ad", [128, NCH, 512], f32, kind="ExternalOutput")

    x_view = x_d[:, :].rearrange("(co ci) s -> ci co s", ci=128)

    with tile.TileContext(nc) as tc, bass.ExitStack() as ctx:
        const = ctx.enter_context(tc.tile_pool(name="const", bufs=1))
        persist = ctx.enter_context(tc.tile_pool(name="persist", bufs=1))
        xpool = ctx.enter_context(tc.tile_pool(name="x", bufs=3))
        thpool = ctx.enter_context(tc.tile_pool(name="th", bufs=4))
        egpool = ctx.enter_context(tc.tile_pool(name="eg", bufs=3))
        grpool = ctx.enter_context(tc.tile_pool(name="gr", bufs=3))
        gbpool = ctx.enter_context(tc.tile_pool(name="gb", bufs=3))
        smalls = ctx.enter_context(tc.tile_pool(name="smalls", bufs=1))
        dram = ctx.enter_context(tc.tile_pool(name="dram", bufs=1, space="DRAM"))

        # ---- constants ----
        wpt_sb = const.tile([128, 2, 256], f32)
        nc.sync.dma_start(wpt_sb[:], wpt_d[:, :, :])
        wg_sb = const.tile([128, 2, 128], f32)
        nc.sync.dma_start(wg_sb[:], wg_d[:, :, :])
        wout_sb = const.tile([128, 2, 128], f32)
        nc.sync.dma_start(wout_sb[:], wout_d[:, :, :])
        bg_sb = const.tile([128, 1], f32)
        nc.sync.dma_start(bg_sb[:], bg_d[:, :])
        bt_sb = const.tile([128, 1], f32)
        nc.sync.dma_start(bt_sb[:], bt_d[:, :])
        bo_sb = const.tile([128, 2], f32)
        nc.sync.dma_start(bo_sb[:], bo_d[:, :])
        ones_sb = const.tile([128, 1], f32)
        nc.vector.memset(ones_sb[:], 1.0)
        ones_row = const.tile([1, 128], f32)
        nc.vector.memset(ones_row[:], 1.0)
        id_sb = const.tile([128, 128], f32)
        make_identity(nc, id_sb[:])

        # ---- persistent big buffers ----
        e_sb = persist.tile([128, NSUB, 128], f32)     # exp(piT), (s, sub, n)
        sg_sb = persist.tile([128, NCH, 512], f32)     # softmax_g, (n, ch, s)

        cc_in = dram.tile([129, 128], f32)
        cc_out = dram.tile([129, 128], f32)

        # ================= phase 1 =================
        with bass.ExitStack() as p1:
            ptp = p1.enter_context(tc.tile_pool(name="ptp", bufs=2, space="PSUM"))
            gp = p1.enter_context(tc.tile_pool(name="gp", bufs=2, space="PSUM"))
            gsp = p1.enter_context(tc.tile_pool(name="gsp", bufs=1, space="PSUM"))
            bcp = p1.enter_context(tc.tile_pool(name="bcp", bufs=1, space="PSUM"))
            accp = p1.enter_context(tc.tile_pool(name="accp", bufs=1, space="PSUM"))

            G_ps = accp.tile([128, 128], f32, tag="G")
            r_ps = accp.tile([1, 128], f32, tag="r")

            for d in range(NDMA):
                x_t = xpool.tile([128, 2, DCH], f32)
                nc.sync.dma_start(x_t[:], x_view[:, :, d * DCH:(d + 1) * DCH])

                # --- transposed pi/theta subchunks ---
                for j in range(DCH // 128):
                    sub = d * (DCH // 128) + j
                    pt_ps = ptp.tile([128, 256], f32)
                    nc.tensor.matmul(
                        pt_ps[:], x_t[:, 0, j * 128:(j + 1) * 128],
                        wpt_sb[:, 0, :], start=True, stop=False)
                    nc.tensor.matmul(
                        pt_ps[:], x_t[:, 1, j * 128:(j + 1) * 128],
                        wpt_sb[:, 1, :], start=False, stop=True)
                    # e = exp(piT)
                    nc.scalar.activation(
                        out=e_sb[:, sub, :], in_=pt_ps[:, 0:128], func=AF.Exp)
                    # thetaT copy to SBUF
                    th_t = thpool.tile([128, 128], f32)
                    nc.vector.tensor_copy(th_t[:], pt_ps[:, 128:256])
                    # G += thetaT^T e ; r += 1^T e
                    nc.tensor.matmul(
                        G_ps[:], th_t[:], e_sb[:, sub, :],
                        start=(sub == 0), stop=(sub == NSUB - 1))
                    nc.tensor.matmul(
                        r_ps[:], ones_sb[:, 0:1], e_sb[:, sub, :],
                        start=(sub == 0), stop=(sub == NSUB - 1))

                # --- natural-layout g path ---
                for h in range(DCH // 512):
                    ch = d * (DCH // 512) + h
                    g_ps = gp.tile([128, 512], f32)
                    nc.tensor.matmul(
                        g_ps[:], wg_sb[:, 0, :], x_t[:, 0, h * 512:(h + 1) * 512],
                        start=True, stop=False)
                    nc.tensor.matmul(
                        g_ps[:], wg_sb[:, 1, :], x_t[:, 1, h * 512:(h + 1) * 512],
                        start=False, stop=True)
                    eg_t = egpool.tile([128, 512], f32)
                    nc.scalar.activation(
                        out=eg_t[:], in_=g_ps[:], func=AF.Exp, bias=bg_sb[:, 0:1])
                    gs_ps = gsp.tile([1, 512], f32)
                    nc.tensor.matmul(
                        gs_ps[:], ones_sb[:, 0:1], eg_t[:], start=True, stop=True)
                    gr_t = grpool.tile([1, 512], f32)
                    nc.vector.reciprocal(gr_t[:], gs_ps[:])
                    # row-broadcast 1/gsum across partitions via K=1 matmul
                    bc_ps = bcp.tile([128, 512], f32)
                    nc.tensor.matmul(
                        bc_ps[:], ones_row[:], gr_t[:], start=True, stop=True)
                    nc.vector.tensor_mul(sg_sb[:, ch, :], eg_t[:], bc_ps[:])

            # ---- evacuate the accumulators, all-reduce across cores ----
            G_sb = smalls.tile([128, 128], f32, tag="G_sb")
            nc.vector.tensor_copy(G_sb[:], G_ps[:])
            r_sb = smalls.tile([1, 128], f32, tag="r_sb")
            nc.vector.tensor_copy(r_sb[:], r_ps[:])

        # alpha_distribute is final: ship it while the collective runs
        nc.sync.dma_start(ad_d[:, :, :], sg_sb[:])

        nc.sync.dma_start(cc_in[0:128, :], G_sb[:])
        nc.sync.dma_start(cc_in[128:129, :], r_sb[:])
        nc.gpsimd.collective_compute(
            "AllReduce",
            mybir.AluOpType.add,
            ins=[cc_in.opt()],
            outs=[cc_out.opt()],
            replica_groups=[list(range(N_CORES))],
        )

        Gg_sb = smalls.tile([128, 128], f32, tag="Gg")
        nc.sync.dma_start(Gg_sb[:], cc_out[0:128, :])
        rrow_sb = smalls.tile([1, 128], f32, tag="rrow")
        nc.sync.dma_start(rrow_sb[:], cc_out[128:129, :])
        rcol_sb = smalls.tile([128, 1], f32, tag="rcol")
        nc.sync.dma_start(rcol_sb[:], cc_out[128:129, :].rearrange("a b -> b a"))

        qcol_sb = smalls.tile([128, 1], f32, tag="qcol")
        nc.vector.reciprocal(qcol_sb[:], rcol_sb[:])
        qrow_sb = smalls.tile([1, 128], f32, tag="qrow")
        nc.vector.reciprocal(qrow_sb[:], rrow_sb[:])

        # ================= phase 2 =================
        with bass.ExitStack() as p2:
            tp = p2.enter_context(tc.tile_pool(name="tp", bufs=1, space="PSUM"))
            zp = p2.enter_context(tc.tile_pool(name="zp", bufs=2, space="PSUM"))
            op = p2.enter_context(tc.tile_pool(name="op", bufs=4, space="PSUM"))
            ztp = p2.enter_context(tc.tile_pool(name="zt", bufs=3))
            outp = p2.enter_context(tc.tile_pool(name="outp", bufs=3))

            # G0^T = (G_allreduced)^T scaled by q = 1/r per row
            GT_ps = tp.tile([128, 128], f32, tag="GT_ps")
            nc.tensor.transpose(GT_ps[:], Gg_sb[:], id_sb[:])
            GT_sb = smalls.tile([128, 128], f32, tag="GT")
            nc.vector.tensor_copy(GT_sb[:], GT_ps[:])
            nc.vector.tensor_scalar_mul(GT_sb[:], in0=GT_sb[:], scalar1=qcol_sb[:])

            # q row broadcast to all partitions (for alpha_gathering scaling)
            qb_ps = tp.tile([128, 128], f32, tag="qb_ps")
            nc.tensor.matmul(
                qb_ps[:], ones_row[:], qrow_sb[:], start=True, stop=True)
            qb_sb = smalls.tile([128, 128], f32, tag="qb")
            nc.vector.tensor_copy(qb_sb[:], qb_ps[:])

            # alpha_gathering: normalize e in place, ship per DMA-chunk
            for d in range(NDMA):
                for j in range(DCH // 128):
                    sub = d * (DCH // 128) + j
                    nc.gpsimd.tensor_mul(
                        e_sb[:, sub, :], e_sb[:, sub, :], qb_sb[:])
                lo, hi = d * (DCH // 128), (d + 1) * (DCH // 128)
                nc.sync.dma_start(
                    ag_d[lo:hi, :, :].rearrange("c s n -> s c n"),
                    e_sb[:, lo:hi, :])

            # Z = G0^T^T sg + bt ; out = Wo Z + bo
            for ch in range(NCH):
                z_ps = zp.tile([128, 512], f32)
                nc.tensor.matmul(
                    z_ps[:], GT_sb[:], sg_sb[:, ch, :], start=True, stop=True)
                z_t = ztp.tile([128, 512], f32)
                nc.scalar.activation(
                    out=z_t[:], in_=z_ps[:], func=AF.Identity, bias=bt_sb[:, 0:1])
                out_t = outp.tile([128, 2, 512], f32)
                for oo in range(2):
                    o_ps = op.tile([128, 512], f32)
                    nc.tensor.matmul(
                        o_ps[:], wout_sb[:, oo, :], z_t[:], start=True, stop=True)
                    nc.scalar.activation(
                        out=out_t[:, oo, :], in_=o_ps[:], func=AF.Identity,
                        bias=bo_sb[:, oo:oo + 1])
                nc.sync.dma_start(
                    out_d[:, :, ch * 512:(ch + 1) * 512], out_t[:])

    _split_excess_waits(nc)
    return nc


_NC = None


def _get_nc():
    global _NC
    if _NC is None:
        _NC = _build_nc()
    return _NC


def kernel(content, style, w_theta, b_theta, w_pi, b_pi, w_g, b_g, w_out, b_out,
           _return_bass_results=False, _trace=False, _tmpdir=None):
    content = np.asarray(content, dtype=np.float32)
    style = np.asarray(style, dtype=np.float32)

    # x in (channel, global-spatial) layout; spatial = (image, h*w) flattened
    x_glob = np.concatenate(
        [content.reshape(C_IN, HW), style.reshape(C_IN, HW)], axis=1)

    # stacked, transposed, c-tiled weights
    wpt = np.ascontiguousarray(
        np.concatenate([np.asarray(w_pi).T, np.asarray(w_theta).T], axis=1)
        .reshape(2, 128, 256).transpose(1, 0, 2))                 # (ci, co, 256)
    wg = np.ascontiguousarray(
        np.asarray(w_g).T.reshape(2, 128, 128).transpose(1, 0, 2))  # (ci, co, n)
    wout = np.ascontiguousarray(np.asarray(w_out).T.reshape(128, 2, 128))
    bg = np.ascontiguousarray(np.asarray(b_g).reshape(128, 1))
    bt = np.ascontiguousarray(np.asarray(b_theta).reshape(128, 1))
    bo = np.ascontiguousarray(np.asarray(b_out).reshape(2, 128).T)

    in_maps = []
    for c in range(N_CORES):
        in_maps.append({
            "x": np.ascontiguousarray(x_glob[:, c * SSH:(c + 1) * SSH]),
            "wpt": wpt, "wg": wg, "wout": wout,
            "bg": bg, "bt": bt, "bo": bo,
        })

    nc = _get_nc()
    kw = {}
    if _trace:
        kw = dict(trace=True, tmpdir=_tmpdir)
    res = run_bass_kernel_spmd(nc, in_maps, core_ids=list(range(N_CORES)), **kw)

    outs = [res.results[c] for c in range(N_CORES)]
    out_glob = np.concatenate(
        [o["out"].transpose(1, 0, 2).reshape(C_IN, SSH) for o in outs], axis=1)
    alpha_g = np.concatenate(
        [o["ag"].reshape(SSH, 128) for o in outs], axis=0)
    alpha_d = np.concatenate(
        [o["ad"].reshape(128, SSH) for o in outs], axis=1)

    content_update = np.ascontiguousarray(out_glob[:, :HW]).reshape(1, C_IN, H, W)
    style_update = np.ascontiguousarray(out_glob[:, HW:]).reshape(1, C_IN, H, W)
    alpha_gathering = alpha_g.reshape(1, K, C_ATT, H, W)
    alpha_distribute = alpha_d.reshape(1, K, C_ATT, H, W)

    ret = (content_update, style_update, alpha_gathering, alpha_distribute)
    if _return_bass_results:
        return ret, res
    return ret


# revision 16
# speedup vs baseline: 1.7329x; 1.7329x over previous
"""A^2-style sparse attention (double-attention) kernel for 8 Trainium2 cores.

Computation (per the reference):
  x = concat(content, style)              (2, 256, 256, 256) -> (256, S), S = 131072
  theta = Wt x + bt ; pi = Wp x + bp ; g = Wg x + bg        (128, S) each
  softmax_pi = softmax(pi, axis=S per channel)              (gathering weights)
  softmax_g  = softmax(g, axis=channels per pixel)          (distribution weights)
  G = theta @ softmax_pi^T ; Z = G @ softmax_g ; out = Wo Z + bo

Sharding: spatial axis S split across 8 cores (16384 columns each).
Device-side restructuring (validated vs reference):
  * softmax_pi is invariant to bp -> bp dropped; no max subtraction in either
    softmax (|pi|,|g| <= ~3.3, exp safe in fp32).
  * pi/theta computed directly transposed, (s, n)-layout, using the x chunk as
    the matmul stationary operand: piT = x_chunk^T @ [WpT | WtT].
  * GT[n, m] = sum_s e[s,n] thetaT[s,m] accumulated in PSUM with a fused ones
    column, so column 128 of the same accumulator is r[n] = sum_s e[s,n].
  * one 128x129 AllReduce of [GT | r]; theta/out biases fold algebraically
    (softmax columns sum to 1) into the Z / out PSUM-evacuation biases.
  * alpha_gathering is written unnormalized (= e); the host multiplies by the
    returned global 1/r row (same DRAM bytes either way).
  * g softmax in natural (n, s) layout; per-pixel channel sums via a
    ones-vector matmul; 1/sums row-broadcast via a K=1 ones matmul.
  * all matmuls run as float32r (single-pass fp32 on the PE, 4x over the
    LOW_HIGH two-pass fp32 mode).
"""

import numpy as np

import concourse.bass as bass
import concourse.tile as tile
from concourse import mybir
from concourse.bass_utils import run_bass_kernel_spmd
from concourse.vector_clock import ScopedClock

f32 = mybir.dt.float32
f32r = mybir.dt.float32r
AF = mybir.ActivationFunctionType

C_IN = 256
C_ATT = 128
K = 2
H = W = 256
HW = H * W
S = K * HW
N_CORES = 8
SSH = S // N_CORES          # 16384 spatial columns per core
NSUB = SSH // 128           # 128 transposed subchunks per core
NCH = SSH // 512            # 32 natural 512-chunks per core
DCH = 2048                  # input DMA chunk (columns)
NDMA = SSH // DCH

# fp32r scope: IN covers the x/weight operands of the projection matmuls
# (biggest PE win); STORE additionally keeps eg/Z/GT/sg in fp32r so the
# gsum/Z/out matmuls also run single-pass.
F32R_IN = True
F32R_STORE = True
DT_IN = f32r if F32R_IN else f32
DT_ST = f32r if F32R_STORE else f32


# ---------------------------------------------------------------------------
# Workaround: walrus in this container accepts only ONE sem wait per
# instruction.  (1) the TileContext exit drain gets its waits split across
# nops; (2) a post-pass splits excess waits everywhere else.
def _patched_drain_and_barrier(self, tick_clock, wait_clock):
    nc = self.nc
    absorbers = [nc.sync.nop(nofuse=True) for _ in range(32)]
    drain_inst = nc.sync.drain()
    wait_clock.add_sem_waits(
        drain_inst.ins, ScopedClock({None: tick_clock.global_clock})
    )
    si = drain_inst.ins.sync_info
    waits = list(si.on_wait) if (si is not None and si.on_wait) else []
    if len(waits) > 1:
        si.on_wait = waits[-1:]
        for ab, w in zip(absorbers, waits[:-1]):
            asi = ab.ins.sync_info
            if asi is None:
                ab.ins.sync_info = type(si)(on_wait=[w], on_update=[])
            else:
                asi.on_wait = list(asi.on_wait or []) + [w]
    nc.all_engine_barrier()
    assert self.sems is not None
    popped = nc._tile_sem_poison_stack.pop()
    assert popped is self._sem_poison
    nc.clear_and_free_semaphores(list(self.sems.allocated().values()))
    nc.all_engine_barrier()


tile.TileContext._drain_and_barrier = _patched_drain_and_barrier


def _split_excess_waits(nc, limit=1):
    n_fix = 0
    for bb in nc.main_func.blocks:
        out = []
        for ins in bb.instructions:
            si = ins.sync_info
            waits = list(si.on_wait) if (si is not None and si.on_wait) else []
            if len(waits) > limit:
                si.on_wait = waits[-limit:]
                for w in waits[:-limit]:
                    n_fix += 1
                    nop = mybir.InstNoOp(
                        name=f"I-waitfix-{n_fix}",
                        engine=ins.engine,
                        sync_info=mybir.SyncInfo(on_wait=[w], on_update=[]),
                    )
                    out.append(nop)
            out.append(ins)
        bb.instructions = out
    return n_fix
# ---------------------------------------------------------------------------


def _build_nc():
    nc = bass.Bass()

    # ---- per-core DRAM I/O ----
    x_d = nc.dram_tensor("x", [C_IN, SSH], DT_IN, kind="ExternalInput")
    wpt_d = nc.dram_tensor("wpt", [128, 2, 256], DT_IN, kind="ExternalInput")
    wg_d = nc.dram_tensor("wg", [128, 2, 128], DT_IN, kind="ExternalInput")
    wout_d = nc.dram_tensor("wout", [128, 2, 128], DT_ST, kind="ExternalInput")
    bg_d = nc.dram_tensor("bg", [128, 1], f32, kind="ExternalInput")
    bt_d = nc.dram_tensor("bt", [128, 1], f32, kind="ExternalInput")
    bo_d = nc.dram_tensor("bo", [128, 2], f32, kind="ExternalInput")

    out_d = nc.dram_tensor("out", [128, 2, SSH], f32, kind="ExternalOutput")
    ag_d = nc.dram_tensor("ag", [NSUB, 128, 128], f32, kind="ExternalOutput")
    ad_d = nc.dram_tensor("ad", [128, NCH, 512], f32, kind="ExternalOutput")
    rg_d = nc.dram_tensor("rg", [128, 1], f32, kind="ExternalOutput")

    x_view = x_d[:, :].rearrange("(co ci) s -> ci co s", ci=128)

    with tile.TileContext(nc) as tc, bass.ExitStack() as ctx:
        const = ctx.enter_context(tc.tile_pool(name="const", bufs=1))
        sgpool = ctx.enter_context(tc.tile_pool(name="sgp", bufs=1))
        thpool = ctx.enter_context(tc.tile_pool(name="th", bufs=4))
        egpool = ctx.enter_context(tc.tile_pool(name="eg", bufs=3))
        grpool = ctx.enter_context(tc.tile_pool(name="gr", bufs=3))
        smalls = ctx.enter_context(tc.tile_pool(name="smalls", bufs=1))
        dram = ctx.enter_context(tc.tile_pool(name="dram", bufs=1, space="DRAM"))

        # ---- constants ----
        wpt_sb = const.tile([128, 2, 256], DT_IN)
        nc.sync.dma_start(wpt_sb[:], wpt_d[:, :, :])
        wg_sb = const.tile([128, 2, 128], DT_IN)
        nc.sync.dma_start(wg_sb[:], wg_d[:, :, :])
        wout_sb = const.tile([128, 2, 128], DT_ST)
        nc.sync.dma_start(wout_sb[:], wout_d[:, :, :])
        bg_sb = const.tile([128, 1], f32)
        nc.sync.dma_start(bg_sb[:], bg_d[:, :])
        bt_sb = const.tile([128, 1], f32)
        nc.sync.dma_start(bt_sb[:], bt_d[:, :])
        bo_sb = const.tile([128, 2], f32)
        nc.sync.dma_start(bo_sb[:], bo_d[:, :])
        ones_sb = const.tile([128, 1], f32)
        nc.vector.memset(ones_sb[:], 1.0)
        ones_row = const.tile([1, 128], f32)
        nc.vector.memset(ones_row[:], 1.0)

        sg_sb = sgpool.tile([128, NCH, 512], DT_ST)      # softmax_g, (n, ch, s)

        cc_in = dram.tile([128, 129], f32)
        cc_out = dram.tile([128, 129], f32)
        wu_in = dram.tile([1, 128], f32)
        wu_out = dram.tile([1, 128], f32)

        # warm up the collective firmware early, overlapped with phase 1
        wu_sb = smalls.tile([1, 128], f32, tag="wu")
        nc.vector.memset(wu_sb[:], 0.0)
        nc.sync.dma_start(wu_in[:], wu_sb[:])
        nc.gpsimd.collective_compute(
            "AllReduce",
            mybir.AluOpType.add,
            ins=[wu_in.opt()],
            outs=[wu_out.opt()],
            replica_groups=[list(range(N_CORES))],
        )

        # ================= phase 1 =================
        with bass.ExitStack() as p1:
            epool = p1.enter_context(tc.tile_pool(name="ep", bufs=1))
            xpool = p1.enter_context(tc.tile_pool(name="x", bufs=2))
            ptp = p1.enter_context(tc.tile_pool(name="ptp", bufs=3, space="PSUM"))
            gp = p1.enter_context(tc.tile_pool(name="gp", bufs=2, space="PSUM"))
            gsp = p1.enter_context(tc.tile_pool(name="gsp", bufs=1, space="PSUM"))
            bcp = p1.enter_context(tc.tile_pool(name="bcp", bufs=1, space="PSUM"))
            accp = p1.enter_context(tc.tile_pool(name="accp", bufs=1, space="PSUM"))

            e_sb = epool.tile([128, NSUB, 128], f32)   # exp(piT), (s, sub, n)
            gt_ps = accp.tile([128, 129], f32)         # [GT | r] accumulator

            for d in range(NDMA):
                x_t = xpool.tile([128, 2, DCH], DT_IN)
                nc.sync.dma_start(x_t[:], x_view[:, :, d * DCH:(d + 1) * DCH])

                # --- transposed pi/theta, two subchunks share one PSUM bank ---
                for jj in range(DCH // 256):
                    sub0 = d * (DCH // 128) + 2 * jj
                    pt_ps = ptp.tile([128, 512], f32)
                    ptv = pt_ps[:].rearrange("p (j t b) -> p j t b", j=2, t=2)
                    for j in range(2):
                        xs = x_t[:, :, (2 * jj + j) * 128:(2 * jj + j + 1) * 128]
                        nc.tensor.matmul(
                            pt_ps[:, j * 256:(j + 1) * 256], xs[:, 0, :],
                            wpt_sb[:, 0, :], start=True, stop=False)
                        nc.tensor.matmul(
                            pt_ps[:, j * 256:(j + 1) * 256], xs[:, 1, :],
                            wpt_sb[:, 1, :], start=False, stop=True)
                    # e = exp(piT) for both subchunks in one pass
                    nc.scalar.activation(
                        out=e_sb[:, sub0:sub0 + 2, :], in_=ptv[:, :, 0, :],
                        func=AF.Exp)
                    # thetaT copy + fused ones column
                    th_t = thpool.tile([128, 2, 129], f32)
                    nc.vector.tensor_copy(th_t[:, :, 0:128], ptv[:, :, 1, :])
                    nc.gpsimd.memset(th_t[:, :, 128:129], 1.0)
                    # [GT | r] += e^T [thetaT | 1]
                    for j in range(2):
                        sub = sub0 + j
                        nc.tensor.matmul(
                            gt_ps[:], e_sb[:, sub, :], th_t[:, j, :],
                            start=(sub == 0), stop=(sub == NSUB - 1))

                # --- natural-layout g path ---
                for h in range(DCH // 512):
                    ch = d * (DCH // 512) + h
                    g_ps = gp.tile([128, 512], f32)
                    nc.tensor.matmul(
                        g_ps[:], wg_sb[:, 0, :],
                        x_t[:, 0, h * 512:(h + 1) * 512],
                        start=True, stop=False)
                    nc.tensor.matmul(
                        g_ps[:], wg_sb[:, 1, :],
                        x_t[:, 1, h * 512:(h + 1) * 512],
                        start=False, stop=True)
                    eg_t = egpool.tile([128, 512], f32)
                    nc.scalar.activation(
                        out=eg_t[:], in_=g_ps[:], func=AF.Exp, bias=bg_sb[:, 0:1])
                    gs_ps = gsp.tile([1, 512], f32)
                    nc.tensor.matmul(
                        gs_ps[:], ones_sb[:, 0:1], eg_t[:],
                        start=True, stop=True)
                    gr_t = grpool.tile([1, 512], f32)
                    nc.vector.reciprocal(gr_t[:], gs_ps[:])
                    bc_ps = bcp.tile([128, 512], f32)
                    nc.tensor.matmul(
                        bc_ps[:], ones_row[:], gr_t[:],
                        start=True, stop=True)
                    nc.vector.tensor_mul(sg_sb[:, ch, :], eg_t[:], bc_ps[:])

                # ship this chunk's unnormalized alpha_gathering
                lo, hi = d * (DCH // 128), (d + 1) * (DCH // 128)
                nc.sync.dma_start(
                    ag_d[lo:hi, :, :].rearrange("c s n -> s c n"),
                    e_sb[:, lo:hi, :])

            # evacuate [GT | r] and all-reduce across cores
            gtr_sb = smalls.tile([128, 129], f32, tag="gtr")
            nc.vector.tensor_copy(gtr_sb[:], gt_ps[:])

        # alpha_distribute is final: ship it while the collective runs
        nc.sync.dma_start(ad_d[:, :, :], sg_sb[:].bitcast(f32))

        nc.sync.dma_start(cc_in[:, :], gtr_sb[:])
        nc.gpsimd.collective_compute(
            "AllReduce",
            mybir.AluOpType.add,
            ins=[cc_in.opt()],
            outs=[cc_out.opt()],
            replica_groups=[list(range(N_CORES))],
        )

        gtg_sb = smalls.tile([128, 129], f32, tag="gtg")
        nc.sync.dma_start(gtg_sb[:], cc_out[:, :])
        nc.sync.dma_start(rg_d[:, :], gtg_sb[:, 128:129])
        qcol_sb = smalls.tile([128, 1], f32, tag="qcol")
        nc.vector.reciprocal(qcol_sb[:], gtg_sb[:, 128:129])
        GT_sb = smalls.tile([128, 128], DT_ST, tag="GT")
        nc.vector.tensor_scalar_mul(
            GT_sb[:], in0=gtg_sb[:, 0:128], scalar1=qcol_sb[:])

        # ================= phase 2 =================
        with bass.ExitStack() as p2:
            zpool = p2.enter_context(tc.tile_pool(name="zpool", bufs=1))
            outp = p2.enter_context(tc.tile_pool(name="outp", bufs=3))
            zp = p2.enter_context(tc.tile_pool(name="zp", bufs=3, space="PSUM"))
            op = p2.enter_context(tc.tile_pool(name="op", bufs=4, space="PSUM"))

            z_sb = zpool.tile([128, NCH, 512], DT_ST)

            # all Z first: the stationary GT loads once
            for ch in range(NCH):
                z_ps = zp.tile([128, 512], f32)
                nc.tensor.matmul(
                    z_ps[:], GT_sb[:], sg_sb[:, ch, :],
                    start=True, stop=True)
                nc.scalar.activation(
                    out=z_sb[:, ch, :], in_=z_ps[:], func=AF.Identity,
                    bias=bt_sb[:, 0:1])

            for ch in range(NCH):
                out_t = outp.tile([128, 2, 512], f32)
                for oo in range(2):
                    o_ps = op.tile([128, 512], f32)
                    nc.tensor.matmul(
                        o_ps[:], wout_sb[:, oo, :], z_sb[:, ch, :],
                        start=True, stop=True)
                    nc.scalar.activation(
                        out=out_t[:, oo, :], in_=o_ps[:], func=AF.Identity,
                        bias=bo_sb[:, oo:oo + 1])
                nc.sync.dma_start(
                    out_d[:, :, ch * 512:(ch + 1) * 512], out_t[:])

    _split_excess_waits(nc)
    return nc


_NC = None


def _get_nc():
    global _NC
    if _NC is None:
        _NC = _build_nc()
    return _NC


def kernel(content, style, w_theta, b_theta, w_pi, b_pi, w_g, b_g, w_out, b_out,
           _return_bass_results=False, _trace=False, _tmpdir=None):
    content = np.asarray(content, dtype=np.float32)
    style = np.asarray(style, dtype=np.float32)

    x_glob = np.concatenate(
        [content.reshape(C_IN, HW), style.reshape(C_IN, HW)], axis=1)

    wpt = np.ascontiguousarray(
        np.concatenate([np.asarray(w_pi).T, np.asarray(w_theta).T], axis=1)
        .reshape(2, 128, 256).transpose(1, 0, 2))                   # (ci, co, 256)
    wg = np.ascontiguousarray(
        np.asarray(w_g).T.reshape(2, 128, 128).transpose(1, 0, 2))  # (ci, co, n)
    wout = np.ascontiguousarray(np.asarray(w_out).T.reshape(128, 2, 128))
    bg = np.ascontiguousarray(np.asarray(b_g).reshape(128, 1))
    bt = np.ascontiguousarray(np.asarray(b_theta).reshape(128, 1))
    bo = np.ascontiguousarray(np.asarray(b_out).reshape(2, 128).T)

    in_maps = []
    for c in range(N_CORES):
        in_maps.append({
            "x": np.ascontiguousarray(x_glob[:, c * SSH:(c + 1) * SSH]),
            "wpt": wpt, "wg": wg, "wout": wout,
            "bg": bg, "bt": bt, "bo": bo,
        })

    nc = _get_nc()
    kw = {}
    if _trace:
        kw = dict(trace=True, tmpdir=_tmpdir)
    res = run_bass_kernel_spmd(nc, in_maps, core_ids=list(range(N_CORES)), **kw)

    outs = [res.results[c] for c in range(N_CORES)]
    out_glob = np.concatenate(
        [o["out"].transpose(1, 0, 2).reshape(C_IN, SSH) for o in outs], axis=1)
    # alpha_gathering: normalize the unnormalized e by the global 1/r
    q = (1.0 / outs[0]["rg"].astype(np.float64)).astype(np.float32)  # (128, 1)
    alpha_g = np.concatenate(
        [o["ag"].reshape(SSH, 128) for o in outs], axis=0)
    alpha_g *= q.reshape(1, 128)
    alpha_d = np.concatenate(
        [o["ad"].reshape(128, SSH) for o in outs], axis=1)

    content_update = np.ascontiguousarray(out_glob[:, :HW]).reshape(1, C_IN, H, W)
    style_update = np.ascontiguousarray(out_glob[:, HW:]).reshape(1, C_IN, H, W)
    alpha_gathering = alpha_g.reshape(1, K, C_ATT, H, W)
    alpha_distribute = alpha_d.reshape(1, K, C_ATT, H, W)

    ret = (content_update, style_update, alpha_gathering, alpha_distribute)
    if _return_bass_results:
        return ret, res
    return ret
